# revision 2
# baseline (speedup 1.0000x reference)
"""Trainium2 Bass kernel for nn_Attention_40407052320989.

Causal GQA attention block (Llama-style): QKV projection + RoPE + causal
softmax attention (8 KV heads, 32 Q heads, n_rep=4) + output projection.

Sharding: tensor-parallel over heads across 8 NeuronCores. Core c owns
KV head c and its 4 query heads: Wq/Wk/Wv column-sharded, Wo row-sharded
by the same head group. Each core computes a full [B, S, D] partial of
the output (its head group's contribution through Wo); the host sums the
8 partials (the row-parallel unshard).

On-chip layout trick: everything is kept "feature on partitions, tokens
on free dim". Scores are computed transposed (scoresT[t, s]) so the
exp'd tile feeds the PV matmul directly as the moving operand, with no
[s,t] -> [t,s] transposes. Softmax sums over t (the partition dim) are
taken with a ones-vector matmul; normalization is broadcast back over
partitions with a rank-1 (K=1) matmul.

RoPE pairing note: Wq/Wk columns are permuted host-side so that rotation
pairs (2i, 2i+1) land at partitions (i, 64+i) (rotate-half layout).
Scores are invariant to any consistent permutation of head-dim columns
of Wq and Wk, so no unpermute is needed. The rotation is then
  out = q * cos2 + swap_halves(q) * sin2n
where cos2 duplicates cos over both partition halves and sin2n carries
[-sin; +sin].

Matmuls run as float32r (fp32 storage, TF32-like multiply): full PE rate
at N>=256, measured ~2e-4 relative error per 128-contraction.
"""

import math
import sys

sys.path.insert(0, "/opt/trn_rl_repo")

import numpy as np

import concourse.bass as bass
import concourse.mybir as mybir
import concourse.tile as tile
from concourse import bacc
from concourse.bass_utils import run_bass_kernel_spmd

F32 = mybir.dt.float32
F32R = mybir.dt.float32r
AF = mybir.ActivationFunctionType

BSZ, SEQLEN, DIM = 2, 2048, 4096
N_HEADS, N_KV_HEADS, HEAD_DIM = 32, 8, 128
N_REP = N_HEADS // N_KV_HEADS  # q heads per core
N_CORES = 8
P = 128
NKD = DIM // P          # 32 contraction chunks for the projections
NT512 = SEQLEN // 512   # 4 blocks of 512 tokens per batch
NTC = SEQLEN // P       # 16 chunks of 128 tokens per batch
SCALE = 1.0 / math.sqrt(HEAD_DIM)

_CACHED = {}


def _build_nc():
    nc = bacc.Bacc(None, target_bir_lowering=False, debug=False)

    xT = nc.declare_dram_parameter("xT", [BSZ, DIM, SEQLEN], F32R, isOutput=False)
    wq = nc.declare_dram_parameter("wq", [DIM, N_REP * HEAD_DIM], F32R, isOutput=False)
    wkv = nc.declare_dram_parameter("wkv", [DIM, 2 * HEAD_DIM], F32R, isOutput=False)
    wo = nc.declare_dram_parameter("wo", [N_REP * HEAD_DIM, DIM], F32R, isOutput=False)
    cos2 = nc.declare_dram_parameter("cos2", [P, SEQLEN], F32R, isOutput=False)
    sin2n = nc.declare_dram_parameter("sin2n", [P, SEQLEN], F32R, isOutput=False)
    dmask = nc.declare_dram_parameter("dmask", [P, 4, 512], F32R, isOutput=False)
    ident = nc.declare_dram_parameter("ident", [P, P], F32R, isOutput=False)
    ones_c = nc.declare_dram_parameter("ones_c", [P, 1], F32R, isOutput=False)
    ones_r = nc.declare_dram_parameter("ones_r", [1, P], F32R, isOutput=False)
    y = nc.declare_dram_parameter("y", [BSZ, SEQLEN, DIM], F32, isOutput=True)

    def ts(i, n):
        return slice(i * n, (i + 1) * n)

    with tile.TileContext(nc) as tc, nc.allow_low_precision(
        reason="float32r tiles are fp32-width; no precision is dropped"
    ):
        with tc.tile_pool(name="const", bufs=1) as cpool:
            cos_sb = cpool.tile([P, SEQLEN], F32R)
            sin_sb = cpool.tile([P, SEQLEN], F32R)
            dm_sb = cpool.tile([P, 4, 512], F32R)
            id_sb = cpool.tile([P, P], F32R)
            onec_sb = cpool.tile([P, 1], F32R)
            oner_sb = cpool.tile([1, P], F32R)
            nc.sync.dma_start(cos_sb[:], cos2[:])
            nc.sync.dma_start(sin_sb[:], sin2n[:])
            nc.sync.dma_start(dm_sb[:], dmask[:])
            nc.sync.dma_start(id_sb[:], ident[:])
            nc.sync.dma_start(onec_sb[:], ones_c[:])
            nc.sync.dma_start(oner_sb[:], ones_r[:])

            for b in range(BSZ):
                _batch(nc, tc, b, xT, wq, wkv, wo, y,
                       cos_sb, sin_sb, dm_sb, id_sb, onec_sb, oner_sb, ts)

    nc.compile()
    return nc


def _rope(nc, pool, out_slice, psum_in, cos_sb, sin_sb, tsl):
    """out = psum_in * cos2 + swap_halves(psum_in) * sin2n, all [128, 512]."""
    qf = pool.tile([P, 512], F32R, tag="rope_qf")
    rot = pool.tile([P, 512], F32R, tag="rope_rot")
    nc.any.tensor_copy(qf[:], psum_in[:])
    nc.sync.dma_start(rot[0:64, :], qf[64:128, :])
    nc.sync.dma_start(rot[64:128, :], qf[0:64, :])
    nc.vector.tensor_mul(out_slice, qf[:], cos_sb[:, tsl])
    nc.vector.tensor_mul(rot[:], rot[:], sin_sb[:, tsl])
    nc.vector.tensor_add(out_slice, out_slice, rot[:])


def _batch(nc, tc, b, xT, wq, wkv, wo, y,
           cos_sb, sin_sb, dm_sb, id_sb, onec_sb, oner_sb, ts):
    with tc.tile_pool(name=f"qkv{b}", bufs=1) as qkvpool:
        qt = qkvpool.tile([P, N_REP, SEQLEN], F32R)   # Q^T per head  [d, s]
        kt = qkvpool.tile([P, SEQLEN], F32R)          # K^T           [d, t]
        vn = qkvpool.tile([P, NTC, HEAD_DIM], F32R)   # V natural     [t, d]

        # ---- Phase 1: projections + RoPE + V transpose ----
        with tc.tile_pool(name=f"w{b}", bufs=1) as wpool, \
             tc.tile_pool(name=f"xs{b}", bufs=4) as xpool, \
             tc.tile_pool(name=f"t{b}", bufs=3) as tpool, \
             tc.tile_pool(name=f"p1_{b}", bufs=1, space="PSUM") as p1:
            wq_sb = wpool.tile([P, NKD, N_REP * HEAD_DIM], F32R)
            wkv_sb = wpool.tile([P, NKD, 2 * HEAD_DIM], F32R)
            wq_r = wq.rearrange("(o p) m -> p o m", p=P)
            wkv_r = wkv.rearrange("(o p) m -> p o m", p=P)
            for o4 in range(4):
                nc.sync.dma_start(wq_sb[:, ts(o4, 8), :], wq_r[:, ts(o4, 8), :])
                nc.sync.dma_start(wkv_sb[:, ts(o4, 8), :], wkv_r[:, ts(o4, 8), :])

            for t5 in range(NT512):
                pq = [p1.tile([P, 512], F32, tag=f"pq{h}", name=f"pq{h}")
                      for h in range(N_REP)]
                pk = p1.tile([P, 512], F32, tag="pk")
                pv = p1.tile([P, 512], F32, tag="pv")
                for kd in range(NKD):
                    xt = xpool.tile([P, 512], F32R, tag="xt")
                    nc.sync.dma_start(xt[:], xT[b, ts(kd, P), ts(t5, 512)])
                    st, sp = kd == 0, kd == NKD - 1
                    for h in range(N_REP):
                        nc.tensor.matmul(pq[h][:], wq_sb[:, kd, ts(h, P)], xt[:],
                                         start=st, stop=sp)
                    nc.tensor.matmul(pk[:], wkv_sb[:, kd, 0:P], xt[:],
                                     start=st, stop=sp)
                    nc.tensor.matmul(pv[:], wkv_sb[:, kd, P:2 * P], xt[:],
                                     start=st, stop=sp)
                tsl = ts(t5, 512)
                _rope(nc, tpool, kt[:, tsl], pk, cos_sb, sin_sb, tsl)
                for h in range(N_REP):
                    _rope(nc, tpool, qt[:, h, tsl], pq[h], cos_sb, sin_sb, tsl)
                # V^T [d, t] -> V natural [t, d] via PE transpose
                vt_tmp = tpool.tile([P, 512], F32R, tag="vt_tmp")
                nc.any.tensor_copy(vt_tmp[:], pv[:])
                for j in range(4):
                    pvt = p1.tile([P, P], F32R, tag="pvt")
                    nc.tensor.transpose(pvt[:], vt_tmp[:, ts(j, P)], id_sb[:])
                    nc.any.tensor_copy(vn[:, t5 * 4 + j, :], pvt[:])

        # ---- Phase 2: causal attention, scores transposed [t, s] ----
        with tc.tile_pool(name=f"ao{b}", bufs=1) as aopool, \
             tc.tile_pool(name=f"e{b}", bufs=4) as epool, \
             tc.tile_pool(name=f"n{b}", bufs=2) as npool, \
             tc.tile_pool(name=f"p2_{b}", bufs=1, space="PSUM") as p2:
            ao = aopool.tile([P, N_REP, SEQLEN], F32R)  # attn_out^T [d, s]
            for h in range(N_REP):
                for s5 in range(NT512):
                    po = p2.tile([P, 512], F32, tag="po")
                    pz = p2.tile([1, 512], F32, tag="pz")
                    ssl = ts(s5, 512)
                    ntc = 4 * s5 + 4
                    for tci in range(ntc):
                        ps = p2.tile([P, 512], F32, tag="ps", bufs=3)
                        nc.tensor.matmul(ps[:], kt[:, ts(tci, P)], qt[:, h, ssl],
                                         start=True, stop=True)
                        ex = epool.tile([P, 512], F32R, tag="ex")
                        nc.scalar.activation(ex[:], ps[:], AF.Exp, scale=SCALE)
                        if tci >= 4 * s5:
                            nc.vector.tensor_mul(ex[:], ex[:],
                                                 dm_sb[:, tci - 4 * s5, :])
                        st, sp = tci == 0, tci == ntc - 1
                        nc.tensor.matmul(po[:], vn[:, tci, :], ex[:],
                                         start=st, stop=sp)
                        nc.tensor.matmul(pz[:], onec_sb[:], ex[:],
                                         start=st, stop=sp)
                    rz = npool.tile([1, 512], F32R, tag="rz")
                    nc.vector.reciprocal(rz[:], pz[:])
                    pb = p2.tile([P, 512], F32, tag="pb")
                    nc.tensor.matmul(pb[:], oner_sb[:], rz[:], start=True, stop=True)
                    rb = npool.tile([P, 512], F32R, tag="rb")
                    nc.any.tensor_copy(rb[:], pb[:])
                    nc.vector.tensor_mul(ao[:, h, ssl], po[:], rb[:])

            # ---- Phase 3: output projection (row-parallel partial) ----
            with tc.tile_pool(name=f"wo{b}", bufs=1) as wopool, \
                 tc.tile_pool(name=f"o{b}", bufs=4) as opool, \
                 tc.tile_pool(name=f"p3_{b}", bufs=2, space="PSUM") as p3:
                wo_sb = wopool.tile([P, N_REP, DIM], F32R)
                wo_r = wo.rearrange("(o p) n -> p o n", p=P)
                for o2 in range(2):
                    nc.sync.dma_start(wo_sb[:, ts(o2, 2), :], wo_r[:, ts(o2, 2), :])
                for s1 in range(NTC):
                    for n5 in range(DIM // 512):
                        pf = p3.tile([P, 512], F32, tag="pf")
                        for dh in range(N_REP):
                            nc.tensor.matmul(pf[:], ao[:, dh, ts(s1, P)],
                                             wo_sb[:, dh, ts(n5, 512)],
                                             start=dh == 0, stop=dh == N_REP - 1)
                        ot = opool.tile([P, 512], F32, tag="ot")
                        nc.any.tensor_copy(ot[:], pf[:])
                        nc.sync.dma_start(y[b, ts(s1, P), ts(n5, 512)], ot[:])


def _prep_inputs(x, freqs_cos, freqs_sin, Wq, Wk, Wv, Wo):
    x = np.ascontiguousarray(np.asarray(x, dtype=np.float32))
    Wq = np.asarray(Wq, dtype=np.float32)
    Wk = np.asarray(Wk, dtype=np.float32)
    Wv = np.asarray(Wv, dtype=np.float32)
    Wo = np.asarray(Wo, dtype=np.float32)
    fc = np.asarray(freqs_cos, dtype=np.float32)
    fs = np.asarray(freqs_sin, dtype=np.float32)

    xT = np.ascontiguousarray(x.transpose(0, 2, 1))  # [B, D, S]

    # rotate-half column permutation within each head
    perm = np.concatenate([np.arange(0, HEAD_DIM, 2), np.arange(1, HEAD_DIM, 2)])

    cos2 = np.concatenate([fc.T, fc.T], axis=0)       # [128, S]
    sin2n = np.concatenate([-fs.T, fs.T], axis=0)     # [128, S]

    # dmask[p, k, j] = 1 if j >= p + 128*k  (valid, t <= s inside diag block)
    jj = np.arange(512)[None, None, :]
    pp = np.arange(P)[:, None, None]
    kk = np.arange(4)[None, :, None]
    dmask = (jj >= pp + P * kk).astype(np.float32)

    ident = np.eye(P, dtype=np.float32)
    ones_c = np.ones((P, 1), np.float32)
    ones_r = np.ones((1, P), np.float32)

    in_maps = []
    for c in range(N_CORES):
        qcols = np.concatenate(
            [(4 * c + h) * HEAD_DIM + perm for h in range(N_REP)])
        kcols = c * HEAD_DIM + perm
        vcols = c * HEAD_DIM + np.arange(HEAD_DIM)
        wq_c = np.ascontiguousarray(Wq[:, qcols])
        wkv_c = np.ascontiguousarray(
            np.concatenate([Wk[:, kcols], Wv[:, vcols]], axis=1))
        wo_c = np.ascontiguousarray(
            Wo[c * N_REP * HEAD_DIM:(c + 1) * N_REP * HEAD_DIM, :])
        in_maps.append({
            "xT": xT, "wq": wq_c, "wkv": wkv_c, "wo": wo_c,
            "cos2": cos2, "sin2n": sin2n, "dmask": dmask,
            "ident": ident, "ones_c": ones_c, "ones_r": ones_r,
        })
    return in_maps


def get_nc():
    if "nc" not in _CACHED:
        _CACHED["nc"] = _build_nc()
    return _CACHED["nc"]


def kernel(x, start_pos, freqs_cos, freqs_sin, mask, cache_k, cache_v,
           Wq, Wk, Wv, Wo, _trace=False, _tmpdir=None):
    assert int(start_pos) == 0, "kernel hardcodes start_pos == 0"
    nc = get_nc()
    in_maps = _prep_inputs(x, freqs_cos, freqs_sin, Wq, Wk, Wv, Wo)
    kwargs = {}
    if _trace:
        kwargs = {"trace": True, "tmpdir": _tmpdir}
    res = run_bass_kernel_spmd(nc, in_maps, core_ids=list(range(N_CORES)), **kwargs)
    out = res.results[0]["y"].astype(np.float64)
    for c in range(1, N_CORES):
        out += res.results[c]["y"]
    out = out.astype(np.float32)
    if _trace:
        return out, res
    return out


# revision 5
# speedup vs baseline: 1.0788x; 1.0788x over previous
"""Trainium2 Bass kernel for nn_Attention_40407052320989.

Causal GQA attention block (Llama-style): QKV projection + RoPE + causal
softmax attention (8 KV heads, 32 Q heads, n_rep=4) + output projection.

Sharding: tensor-parallel over heads across 8 NeuronCores. Core c owns
KV head c and its 4 query heads: Wq/Wk/Wv column-sharded, Wo row-sharded
by the same head group. Each core computes a full [B, S, D] partial of
the output (its head group's contribution through Wo); the host sums the
8 partials (the row-parallel unshard).

On-chip layout: everything is "feature on partitions, tokens on free
dim". Scores are computed transposed (scoresT[t, s]) so the exp'd tile
feeds the PV matmul directly as the moving operand with no transposes.
Softmax sums over t (partition dim) ride a ones-vector matmul; the
normalizer is broadcast back over partitions with a rank-1 matmul and
inverted as a full [128, 512] DVE reciprocal (a [1, 512] reciprocal
runs ~3.3us on one lane).

RoPE pairing: Wq/Wk columns are permuted host-side so rotation pairs
(2i, 2i+1) land at partitions (i, 64+i) (rotate-half layout). Scores
are invariant to a consistent head-dim permutation of Wq and Wk. The
rotation is out = q * cos2 + swap_halves(q) * sin2n with cos2 = [cos;
cos] and sin2n = [-sin; sin]; the halves swap is two SBUF->SBUF DMAs.

Matmul operands are bf16 (measured: f32r runs ~2 cyc/row on HW, bf16 1
cyc/row); PSUM accumulation, softmax normalization, and the output stay
fp32. End-to-end numpy simulation of this precision mix: 4e-3 max rel.
"""

import math
import sys

sys.path.insert(0, "/opt/trn_rl_repo")

import ml_dtypes
import numpy as np

import concourse.bass as bass
import concourse.mybir as mybir
import concourse.tile as tile
from concourse import bacc
from concourse.bass_utils import run_bass_kernel_spmd

F32 = mybir.dt.float32
F32R = mybir.dt.float32r
BF16 = mybir.dt.bfloat16
AF = mybir.ActivationFunctionType
NP_BF16 = ml_dtypes.bfloat16

BSZ, SEQLEN, DIM = 2, 2048, 4096
N_HEADS, N_KV_HEADS, HEAD_DIM = 32, 8, 128
N_REP = N_HEADS // N_KV_HEADS  # q heads per core
N_CORES = 8
P = 128
NKD = DIM // P          # 32 contraction chunks for the projections
NT512 = SEQLEN // 512   # 4 blocks of 512 tokens per batch
NTC = SEQLEN // P       # 16 chunks of 128 tokens per batch
SCALE = 1.0 / math.sqrt(HEAD_DIM)

_CACHED = {}


def _build_nc():
    nc = bacc.Bacc(None, target_bir_lowering=False, debug=False)

    xT = nc.declare_dram_parameter("xT", [BSZ, DIM, SEQLEN], BF16, isOutput=False)
    wq = nc.declare_dram_parameter("wq", [DIM, N_REP * HEAD_DIM], BF16, isOutput=False)
    wkv = nc.declare_dram_parameter("wkv", [DIM, 2 * HEAD_DIM], BF16, isOutput=False)
    wo = nc.declare_dram_parameter("wo", [N_REP * HEAD_DIM, DIM], BF16, isOutput=False)
    cos2 = nc.declare_dram_parameter("cos2", [P, SEQLEN], F32, isOutput=False)
    sin2n = nc.declare_dram_parameter("sin2n", [P, SEQLEN], F32, isOutput=False)
    dmask = nc.declare_dram_parameter("dmask", [P, 4, 512], BF16, isOutput=False)
    ident = nc.declare_dram_parameter("ident", [P, P], BF16, isOutput=False)
    ones_c = nc.declare_dram_parameter("ones_c", [P, 1], BF16, isOutput=False)
    ones_r = nc.declare_dram_parameter("ones_r", [1, P], F32R, isOutput=False)
    y = nc.declare_dram_parameter("y", [BSZ, SEQLEN, DIM], F32, isOutput=True)

    def ts(i, n):
        return slice(i * n, (i + 1) * n)

    with tile.TileContext(nc) as tc, nc.allow_low_precision(
        reason="psum accumulation and normalization stay fp32 by construction"
    ):
        with tc.tile_pool(name="const", bufs=1) as cpool:
            cos_sb = cpool.tile([P, SEQLEN], F32)
            sin_sb = cpool.tile([P, SEQLEN], F32)
            dm_sb = cpool.tile([P, 4, 512], BF16)
            id_sb = cpool.tile([P, P], BF16)
            onec_sb = cpool.tile([P, 1], BF16)
            oner_sb = cpool.tile([1, P], F32R)
            nc.sync.dma_start(cos_sb[:], cos2[:])
            nc.sync.dma_start(sin_sb[:], sin2n[:])
            nc.sync.dma_start(dm_sb[:], dmask[:])
            nc.sync.dma_start(id_sb[:], ident[:])
            nc.sync.dma_start(onec_sb[:], ones_c[:])
            nc.sync.dma_start(oner_sb[:], ones_r[:])

            for b in range(BSZ):
                _batch(nc, tc, b, xT, wq, wkv, wo, y,
                       cos_sb, sin_sb, dm_sb, id_sb, onec_sb, oner_sb, ts)

    nc.compile()
    return nc


def _rope(nc, pool, out_slice, psum_in, cos_sb, sin_sb, tsl):
    """out = psum_in * cos2 + swap_halves(psum_in) * sin2n, [128, 512].

    psum_in is fp32 PSUM; out_slice is bf16 SBUF (cast on the final add).
    """
    qf = pool.tile([P, 512], F32, tag="rope_qf")
    rot = pool.tile([P, 512], F32, tag="rope_rot")
    tmpa = pool.tile([P, 512], F32, tag="rope_tmpa")
    nc.any.tensor_copy(qf[:], psum_in[:])
    nc.sync.dma_start(rot[0:64, :], qf[64:128, :])
    nc.sync.dma_start(rot[64:128, :], qf[0:64, :])
    nc.vector.tensor_mul(tmpa[:], qf[:], cos_sb[:, tsl])
    nc.vector.tensor_mul(rot[:], rot[:], sin_sb[:, tsl])
    nc.vector.tensor_add(out_slice, tmpa[:], rot[:])


def _batch(nc, tc, b, xT, wq, wkv, wo, y,
           cos_sb, sin_sb, dm_sb, id_sb, onec_sb, oner_sb, ts):
    with tc.tile_pool(name=f"qkv{b}", bufs=1) as qkvpool:
        qt = qkvpool.tile([P, N_REP, SEQLEN], BF16)   # Q^T per head  [d, s]
        kt = qkvpool.tile([P, SEQLEN], BF16)          # K^T           [d, t]
        vn = qkvpool.tile([P, NTC, HEAD_DIM], BF16)   # V natural     [t, d]

        # ---- Phase 1: projections + RoPE + V transpose ----
        with tc.tile_pool(name=f"w{b}", bufs=1) as wpool, \
             tc.tile_pool(name=f"xs{b}", bufs=4) as xpool, \
             tc.tile_pool(name=f"t{b}", bufs=3) as tpool, \
             tc.tile_pool(name=f"p1_{b}", bufs=1, space="PSUM") as p1:
            wq_sb = wpool.tile([P, NKD, N_REP * HEAD_DIM], BF16)
            wkv_sb = wpool.tile([P, NKD, 2 * HEAD_DIM], BF16)
            wq_r = wq.rearrange("(o p) m -> p o m", p=P)
            wkv_r = wkv.rearrange("(o p) m -> p o m", p=P)
            for o4 in range(4):
                nc.sync.dma_start(wq_sb[:, ts(o4, 8), :], wq_r[:, ts(o4, 8), :])
                nc.sync.dma_start(wkv_sb[:, ts(o4, 8), :], wkv_r[:, ts(o4, 8), :])

            for t5 in range(NT512):
                pq = [p1.tile([P, 512], F32, tag=f"pq{h}", name=f"pq{h}")
                      for h in range(N_REP)]
                pk = p1.tile([P, 512], F32, tag="pk")
                pv = p1.tile([P, 512], F32, tag="pv")
                for kd in range(NKD):
                    xt = xpool.tile([P, 512], BF16, tag="xt")
                    nc.sync.dma_start(xt[:], xT[b, ts(kd, P), ts(t5, 512)])
                    st, sp = kd == 0, kd == NKD - 1
                    for h in range(N_REP):
                        nc.tensor.matmul(pq[h][:], wq_sb[:, kd, ts(h, P)], xt[:],
                                         start=st, stop=sp)
                    nc.tensor.matmul(pk[:], wkv_sb[:, kd, 0:P], xt[:],
                                     start=st, stop=sp)
                    nc.tensor.matmul(pv[:], wkv_sb[:, kd, P:2 * P], xt[:],
                                     start=st, stop=sp)
                tsl = ts(t5, 512)
                _rope(nc, tpool, kt[:, tsl], pk, cos_sb, sin_sb, tsl)
                for h in range(N_REP):
                    _rope(nc, tpool, qt[:, h, tsl], pq[h], cos_sb, sin_sb, tsl)
                # V^T [d, t] -> V natural [t, d] via PE transpose
                vt_tmp = tpool.tile([P, 512], BF16, tag="vt_tmp")
                nc.any.tensor_copy(vt_tmp[:], pv[:])
                for j in range(4):
                    pvt = p1.tile([P, P], BF16, tag="pvt")
                    nc.tensor.transpose(pvt[:], vt_tmp[:, ts(j, P)], id_sb[:])
                    nc.any.tensor_copy(vn[:, t5 * 4 + j, :], pvt[:])

        # ---- Phase 2: causal attention, scores transposed [t, s] ----
        with tc.tile_pool(name=f"ao{b}", bufs=1) as aopool:
            ao = aopool.tile([P, N_REP, SEQLEN], BF16)  # attn_out^T [d, s]
            _phase2(nc, tc, b, ao, qt, kt, vn, dm_sb, onec_sb, oner_sb, ts)
            _phase3(nc, tc, b, ao, wo, y, ts)


def _phase2(nc, tc, b, ao, qt, kt, vn, dm_sb, onec_sb, oner_sb, ts):
    with tc.tile_pool(name=f"e{b}", bufs=4) as epool, \
         tc.tile_pool(name=f"n{b}", bufs=2) as npool, \
         tc.tile_pool(name=f"p2_{b}", bufs=1, space="PSUM") as p2:
            for h in range(N_REP):
                for s5 in range(NT512):
                    po = p2.tile([P, 512], F32, tag="po", bufs=2)
                    pz = p2.tile([1, 512], F32, tag="pz", bufs=2)
                    ssl = ts(s5, 512)
                    ntc = 4 * s5 + 4
                    for tci in range(ntc):
                        ps = p2.tile([P, 512], F32, tag="ps", bufs=3)
                        nc.tensor.matmul(ps[:], kt[:, ts(tci, P)], qt[:, h, ssl],
                                         start=True, stop=True)
                        ex = epool.tile([P, 512], BF16, tag="ex")
                        nc.scalar.activation(ex[:], ps[:], AF.Exp, scale=SCALE)
                        if tci >= 4 * s5:
                            nc.vector.tensor_mul(ex[:], ex[:],
                                                 dm_sb[:, tci - 4 * s5, :])
                        st, sp = tci == 0, tci == ntc - 1
                        nc.tensor.matmul(po[:], vn[:, tci, :], ex[:],
                                         start=st, stop=sp)
                        nc.tensor.matmul(pz[:], onec_sb[:], ex[:],
                                         start=st, stop=sp)
                    # broadcast sums over partitions, then invert at full width
                    zs = npool.tile([1, 512], F32R, tag="zs")
                    nc.any.tensor_copy(zs[:], pz[:])
                    pb = p2.tile([P, 512], F32, tag="pb")
                    nc.tensor.matmul(pb[:], oner_sb[:], zs[:], start=True, stop=True)
                    rb = npool.tile([P, 512], F32, tag="rb")
                    nc.vector.reciprocal(rb[:], pb[:])
                    nc.vector.tensor_mul(ao[:, h, ssl], po[:], rb[:])


def _phase3(nc, tc, b, ao, wo, y, ts):
    """Output projection (row-parallel partial)."""
    with tc.tile_pool(name=f"wo{b}", bufs=1) as wopool, \
         tc.tile_pool(name=f"o{b}", bufs=4) as opool, \
         tc.tile_pool(name=f"p3_{b}", bufs=2, space="PSUM") as p3:
        wo_sb = wopool.tile([P, N_REP, DIM], BF16)
        wo_r = wo.rearrange("(o p) n -> p o n", p=P)
        for o2 in range(2):
            nc.sync.dma_start(wo_sb[:, ts(o2, 2), :], wo_r[:, ts(o2, 2), :])
        for s1 in range(NTC):
            for n5 in range(DIM // 512):
                pf = p3.tile([P, 512], F32, tag="pf")
                for dh in range(N_REP):
                    nc.tensor.matmul(pf[:], ao[:, dh, ts(s1, P)],
                                     wo_sb[:, dh, ts(n5, 512)],
                                     start=dh == 0, stop=dh == N_REP - 1)
                ot = opool.tile([P, 512], F32, tag="ot")
                nc.any.tensor_copy(ot[:], pf[:])
                nc.sync.dma_start(y[b, ts(s1, P), ts(n5, 512)], ot[:])


def _prep_inputs(x, freqs_cos, freqs_sin, Wq, Wk, Wv, Wo):
    x = np.ascontiguousarray(np.asarray(x, dtype=np.float32))
    Wq = np.asarray(Wq, dtype=np.float32)
    Wk = np.asarray(Wk, dtype=np.float32)
    Wv = np.asarray(Wv, dtype=np.float32)
    Wo = np.asarray(Wo, dtype=np.float32)
    fc = np.asarray(freqs_cos, dtype=np.float32)
    fs = np.asarray(freqs_sin, dtype=np.float32)

    xT = np.ascontiguousarray(x.transpose(0, 2, 1)).astype(NP_BF16)  # [B, D, S]

    # rotate-half column permutation within each head
    perm = np.concatenate([np.arange(0, HEAD_DIM, 2), np.arange(1, HEAD_DIM, 2)])

    cos2 = np.concatenate([fc.T, fc.T], axis=0)       # [128, S]
    sin2n = np.concatenate([-fs.T, fs.T], axis=0)     # [128, S]

    # dmask[p, k, j] = 1 if j >= p + 128*k  (valid, t <= s inside diag block)
    jj = np.arange(512)[None, None, :]
    pp = np.arange(P)[:, None, None]
    kk = np.arange(4)[None, :, None]
    dmask = (jj >= pp + P * kk).astype(NP_BF16)

    ident = np.eye(P, dtype=NP_BF16)
    ones_c = np.ones((P, 1), NP_BF16)
    ones_r = np.ones((1, P), np.float32)

    in_maps = []
    for c in range(N_CORES):
        qcols = np.concatenate(
            [(4 * c + h) * HEAD_DIM + perm for h in range(N_REP)])
        kcols = c * HEAD_DIM + perm
        vcols = c * HEAD_DIM + np.arange(HEAD_DIM)
        wq_c = np.ascontiguousarray(Wq[:, qcols]).astype(NP_BF16)
        wkv_c = np.ascontiguousarray(
            np.concatenate([Wk[:, kcols], Wv[:, vcols]], axis=1)).astype(NP_BF16)
        wo_c = np.ascontiguousarray(
            Wo[c * N_REP * HEAD_DIM:(c + 1) * N_REP * HEAD_DIM, :]).astype(NP_BF16)
        in_maps.append({
            "xT": xT, "wq": wq_c, "wkv": wkv_c, "wo": wo_c,
            "cos2": cos2, "sin2n": sin2n, "dmask": dmask,
            "ident": ident, "ones_c": ones_c, "ones_r": ones_r,
        })
    return in_maps


def get_nc():
    if "nc" not in _CACHED:
        _CACHED["nc"] = _build_nc()
    return _CACHED["nc"]


def kernel(x, start_pos, freqs_cos, freqs_sin, mask, cache_k, cache_v,
           Wq, Wk, Wv, Wo, _trace=False, _tmpdir=None):
    assert int(start_pos) == 0, "kernel hardcodes start_pos == 0"
    nc = get_nc()
    in_maps = _prep_inputs(x, freqs_cos, freqs_sin, Wq, Wk, Wv, Wo)
    kwargs = {}
    if _trace:
        kwargs = {"trace": True, "tmpdir": _tmpdir}
    res = run_bass_kernel_spmd(nc, in_maps, core_ids=list(range(N_CORES)), **kwargs)
    out = res.results[0]["y"].astype(np.float64)
    for c in range(1, N_CORES):
        out += res.results[c]["y"]
    out = out.astype(np.float32)
    if _trace:
        return out, res
    return out


# revision 6
# speedup vs baseline: 1.1952x; 1.1079x over previous
"""Trainium2 Bass kernel for nn_Attention_40407052320989.

Causal GQA attention block (Llama-style): QKV projection + RoPE + causal
softmax attention (8 KV heads, 32 Q heads, n_rep=4) + output projection.

Sharding: tensor-parallel over heads across 8 NeuronCores. Core c owns
KV head c and its 4 query heads: Wq/Wk/Wv column-sharded, Wo row-sharded
by the same head group. Each core computes a full [B, S, D] partial of
the output (its head group's contribution through Wo); the host sums the
8 partials (the row-parallel unshard).

On-chip layout: everything is "feature on partitions, tokens on free
dim". Scores are computed transposed (scoresT[t, s]) so the exp'd tile
feeds the PV matmul directly as the moving operand with no transposes.
Softmax sums over t (partition dim) ride a ones-vector matmul; the
normalizer is broadcast back over partitions with a rank-1 matmul and
inverted as a full [128, 512] DVE reciprocal (a [1, 512] reciprocal
runs ~3.3us on one lane).

RoPE pairing: Wq/Wk columns are permuted host-side so rotation pairs
(2i, 2i+1) land at partitions (i, 64+i) (rotate-half layout). Scores
are invariant to a consistent head-dim permutation of Wq and Wk. The
rotation is out = q * cos2 + swap_halves(q) * sin2n with cos2 = [cos;
cos] and sin2n = [-sin; sin]; the halves swap is two SBUF->SBUF DMAs.

Matmul operands are bf16 (measured: f32r runs ~2 cyc/row on HW, bf16 1
cyc/row); PSUM accumulation, softmax normalization, and the output stay
fp32. End-to-end numpy simulation of this precision mix: 4e-3 max rel.
"""

import math
import sys

sys.path.insert(0, "/opt/trn_rl_repo")

import ml_dtypes
import numpy as np

import concourse.bass as bass
import concourse.mybir as mybir
import concourse.tile as tile
from concourse import bacc
from concourse.bass_utils import run_bass_kernel_spmd

F32 = mybir.dt.float32
F32R = mybir.dt.float32r
BF16 = mybir.dt.bfloat16
AF = mybir.ActivationFunctionType
NP_BF16 = ml_dtypes.bfloat16

BSZ, SEQLEN, DIM = 2, 2048, 4096
N_HEADS, N_KV_HEADS, HEAD_DIM = 32, 8, 128
N_REP = N_HEADS // N_KV_HEADS  # q heads per core
N_CORES = 8
P = 128
NKD = DIM // P          # 32 contraction chunks for the projections
NT512 = SEQLEN // 512   # 4 blocks of 512 tokens per batch
NTC = SEQLEN // P       # 16 chunks of 128 tokens per batch
SCALE = 1.0 / math.sqrt(HEAD_DIM)

_CACHED = {}


def _build_nc():
    nc = bacc.Bacc(None, target_bir_lowering=False, debug=False)

    xT = nc.declare_dram_parameter("xT", [BSZ, DIM, SEQLEN], BF16, isOutput=False)
    wq = nc.declare_dram_parameter("wq", [DIM, N_REP * HEAD_DIM], BF16, isOutput=False)
    wkv = nc.declare_dram_parameter("wkv", [DIM, 2 * HEAD_DIM], BF16, isOutput=False)
    wo = nc.declare_dram_parameter("wo", [N_REP * HEAD_DIM, DIM], BF16, isOutput=False)
    cos2 = nc.declare_dram_parameter("cos2", [P, SEQLEN], F32, isOutput=False)
    sin2n = nc.declare_dram_parameter("sin2n", [P, SEQLEN], F32, isOutput=False)
    dmask = nc.declare_dram_parameter("dmask", [P, 4, 512], BF16, isOutput=False)
    ident = nc.declare_dram_parameter("ident", [P, P], BF16, isOutput=False)
    ones_c = nc.declare_dram_parameter("ones_c", [P, 1], BF16, isOutput=False)
    ones_r = nc.declare_dram_parameter("ones_r", [1, P], F32R, isOutput=False)
    y = nc.declare_dram_parameter("y", [BSZ, SEQLEN, DIM], F32, isOutput=True)

    def ts(i, n):
        return slice(i * n, (i + 1) * n)

    with tile.TileContext(nc) as tc, nc.allow_low_precision(
        reason="psum accumulation and normalization stay fp32 by construction"
    ):
        with tc.tile_pool(name="const", bufs=1) as cpool:
            cos_sb = cpool.tile([P, SEQLEN], F32)
            sin_sb = cpool.tile([P, SEQLEN], F32)
            dm_sb = cpool.tile([P, 4, 512], BF16)
            id_sb = cpool.tile([P, P], BF16)
            onec_sb = cpool.tile([P, 1], BF16)
            oner_sb = cpool.tile([1, P], F32R)
            nc.sync.dma_start(cos_sb[:], cos2[:])
            nc.sync.dma_start(sin_sb[:], sin2n[:])
            nc.sync.dma_start(dm_sb[:], dmask[:])
            nc.sync.dma_start(id_sb[:], ident[:])
            nc.sync.dma_start(onec_sb[:], ones_c[:])
            nc.sync.dma_start(oner_sb[:], ones_r[:])

            with tc.tile_pool(name="wpool", bufs=1) as wpool:
                wq_sb = wpool.tile([P, NKD, N_REP * HEAD_DIM], BF16)
                wkv_sb = wpool.tile([P, NKD, 2 * HEAD_DIM], BF16)
                wo_sb = wpool.tile([P, N_REP, DIM], BF16)
                wq_r = wq.rearrange("(o p) m -> p o m", p=P)
                wkv_r = wkv.rearrange("(o p) m -> p o m", p=P)
                wo_r = wo.rearrange("(o p) n -> p o n", p=P)
                for o4 in range(4):
                    nc.sync.dma_start(wq_sb[:, ts(o4, 8), :], wq_r[:, ts(o4, 8), :])
                    nc.sync.dma_start(wkv_sb[:, ts(o4, 8), :], wkv_r[:, ts(o4, 8), :])
                    nc.sync.dma_start(wo_sb[:, o4, :], wo_r[:, o4, :])

                for b in range(BSZ):
                    _batch(nc, tc, b, xT, wq_sb, wkv_sb, wo_sb, y,
                           cos_sb, sin_sb, dm_sb, id_sb, onec_sb, oner_sb, ts)

    nc.compile()
    return nc


def _rope(nc, pool, out_slice, psum_in, cos_sb, sin_sb, tsl):
    """out = psum_in * cos2 + swap_halves(psum_in) * sin2n, [128, 512].

    psum_in is fp32 PSUM; out_slice is bf16 SBUF (cast on the final add).
    """
    qf = pool.tile([P, 512], F32, tag="rope_qf")
    rot = pool.tile([P, 512], F32, tag="rope_rot")
    tmpa = pool.tile([P, 512], F32, tag="rope_tmpa")
    nc.any.tensor_copy(qf[:], psum_in[:])
    nc.sync.dma_start(rot[0:64, :], qf[64:128, :])
    nc.sync.dma_start(rot[64:128, :], qf[0:64, :])
    nc.vector.tensor_mul(tmpa[:], qf[:], cos_sb[:, tsl])
    nc.vector.tensor_mul(rot[:], rot[:], sin_sb[:, tsl])
    nc.vector.tensor_add(out_slice, tmpa[:], rot[:])


def _batch(nc, tc, b, xT, wq_sb, wkv_sb, wo_sb, y,
           cos_sb, sin_sb, dm_sb, id_sb, onec_sb, oner_sb, ts):
    with tc.tile_pool(name=f"qkv{b}", bufs=1) as qkvpool:
        qt = qkvpool.tile([P, N_REP, SEQLEN], BF16)   # Q^T per head  [d, s]
        kt = qkvpool.tile([P, SEQLEN], BF16)          # K^T           [d, t]
        vn = qkvpool.tile([P, NTC, HEAD_DIM], BF16)   # V natural     [t, d]

        # ---- Phase 1: projections + RoPE + V transpose ----
        with tc.tile_pool(name=f"xs{b}", bufs=6) as xpool, \
             tc.tile_pool(name=f"t{b}", bufs=3) as tpool, \
             tc.tile_pool(name=f"p1_{b}", bufs=1, space="PSUM") as p1:
            for t5 in range(NT512):
                pq = [p1.tile([P, 512], F32, tag=f"pq{h}", name=f"pq{h}")
                      for h in range(N_REP)]
                pk = p1.tile([P, 512], F32, tag="pk")
                pv = p1.tile([P, 512], F32, tag="pv")
                for kd in range(NKD):
                    xt = xpool.tile([P, 512], BF16, tag="xt")
                    nc.sync.dma_start(xt[:], xT[b, ts(kd, P), ts(t5, 512)])
                    st, sp = kd == 0, kd == NKD - 1
                    for h in range(N_REP):
                        nc.tensor.matmul(pq[h][:], wq_sb[:, kd, ts(h, P)], xt[:],
                                         start=st, stop=sp)
                    nc.tensor.matmul(pk[:], wkv_sb[:, kd, 0:P], xt[:],
                                     start=st, stop=sp)
                    nc.tensor.matmul(pv[:], wkv_sb[:, kd, P:2 * P], xt[:],
                                     start=st, stop=sp)
                tsl = ts(t5, 512)
                _rope(nc, tpool, kt[:, tsl], pk, cos_sb, sin_sb, tsl)
                for h in range(N_REP):
                    _rope(nc, tpool, qt[:, h, tsl], pq[h], cos_sb, sin_sb, tsl)
                # V^T [d, t] -> V natural [t, d] via PE transpose
                vt_tmp = tpool.tile([P, 512], BF16, tag="vt_tmp")
                nc.any.tensor_copy(vt_tmp[:], pv[:])
                for j in range(4):
                    pvt = p1.tile([P, P], BF16, tag="pvt")
                    nc.tensor.transpose(pvt[:], vt_tmp[:, ts(j, P)], id_sb[:])
                    nc.any.tensor_copy(vn[:, t5 * 4 + j, :], pvt[:])

        # ---- Phase 2: causal attention, scores transposed [t, s] ----
        with tc.tile_pool(name=f"ao{b}", bufs=1) as aopool:
            ao = aopool.tile([P, N_REP, SEQLEN], BF16)  # attn_out^T [d, s]
            _phase2(nc, tc, b, ao, qt, kt, vn, dm_sb, onec_sb, oner_sb, ts)
            _phase3(nc, tc, b, ao, wo_sb, y, ts)


def _phase2(nc, tc, b, ao, qt, kt, vn, dm_sb, onec_sb, oner_sb, ts):
    with tc.tile_pool(name=f"e{b}", bufs=4) as epool, \
         tc.tile_pool(name=f"n{b}", bufs=2) as npool, \
         tc.tile_pool(name=f"p2_{b}", bufs=1, space="PSUM") as p2:
            for h in range(N_REP):
                for s5 in range(NT512):
                    po = p2.tile([P, 512], F32, tag="po", bufs=2)
                    pz = p2.tile([1, 512], F32, tag="pz", bufs=2)
                    ssl = ts(s5, 512)
                    ntc = 4 * s5 + 4
                    for tci in range(ntc):
                        ps = p2.tile([P, 512], F32, tag="ps", bufs=3)
                        nc.tensor.matmul(ps[:], kt[:, ts(tci, P)], qt[:, h, ssl],
                                         start=True, stop=True)
                        ex = epool.tile([P, 512], BF16, tag="ex")
                        nc.scalar.activation(ex[:], ps[:], AF.Exp, scale=SCALE)
                        if tci >= 4 * s5:
                            nc.vector.tensor_mul(ex[:], ex[:],
                                                 dm_sb[:, tci - 4 * s5, :])
                        st, sp = tci == 0, tci == ntc - 1
                        nc.tensor.matmul(po[:], vn[:, tci, :], ex[:],
                                         start=st, stop=sp)
                        nc.tensor.matmul(pz[:], onec_sb[:], ex[:],
                                         start=st, stop=sp)
                    # broadcast sums over partitions, then invert at full width
                    zs = npool.tile([1, 512], F32R, tag="zs")
                    nc.vector.tensor_copy(zs[:], pz[:])
                    pb = p2.tile([P, 512], F32, tag="pb")
                    nc.tensor.matmul(pb[:], oner_sb[:], zs[:], start=True, stop=True)
                    rb = npool.tile([P, 512], F32, tag="rb")
                    nc.vector.reciprocal(rb[:], pb[:])
                    nc.vector.tensor_mul(ao[:, h, ssl], po[:], rb[:])


def _phase3(nc, tc, b, ao, wo_sb, y, ts):
    """Output projection (row-parallel partial)."""
    with tc.tile_pool(name=f"o{b}", bufs=3) as opool, \
         tc.tile_pool(name=f"p3_{b}", bufs=2, space="PSUM") as p3:
        for s1 in range(NTC):
            for half in range(2):
                pf = p3.tile([P, 4, 512], F32, tag="pf")
                for nq in range(4):
                    n5 = half * 4 + nq
                    for dh in range(N_REP):
                        nc.tensor.matmul(pf[:, nq, :], ao[:, dh, ts(s1, P)],
                                         wo_sb[:, dh, ts(n5, 512)],
                                         start=dh == 0, stop=dh == N_REP - 1)
                ot = opool.tile([P, 2048], F32, tag="ot")
                nc.vector.tensor_copy(ot[:], pf[:])
                nc.sync.dma_start(y[b, ts(s1, P), ts(half, 2048)], ot[:])


def _prep_inputs(x, freqs_cos, freqs_sin, Wq, Wk, Wv, Wo):
    x = np.ascontiguousarray(np.asarray(x, dtype=np.float32))
    Wq = np.asarray(Wq, dtype=np.float32)
    Wk = np.asarray(Wk, dtype=np.float32)
    Wv = np.asarray(Wv, dtype=np.float32)
    Wo = np.asarray(Wo, dtype=np.float32)
    fc = np.asarray(freqs_cos, dtype=np.float32)
    fs = np.asarray(freqs_sin, dtype=np.float32)

    xT = np.ascontiguousarray(x.transpose(0, 2, 1)).astype(NP_BF16)  # [B, D, S]

    # rotate-half column permutation within each head
    perm = np.concatenate([np.arange(0, HEAD_DIM, 2), np.arange(1, HEAD_DIM, 2)])

    cos2 = np.concatenate([fc.T, fc.T], axis=0)       # [128, S]
    sin2n = np.concatenate([-fs.T, fs.T], axis=0)     # [128, S]

    # dmask[p, k, j] = 1 if j >= p + 128*k  (valid, t <= s inside diag block)
    jj = np.arange(512)[None, None, :]
    pp = np.arange(P)[:, None, None]
    kk = np.arange(4)[None, :, None]
    dmask = (jj >= pp + P * kk).astype(NP_BF16)

    ident = np.eye(P, dtype=NP_BF16)
    ones_c = np.ones((P, 1), NP_BF16)
    ones_r = np.ones((1, P), np.float32)

    in_maps = []
    for c in range(N_CORES):
        qcols = np.concatenate(
            [(4 * c + h) * HEAD_DIM + perm for h in range(N_REP)])
        kcols = c * HEAD_DIM + perm
        vcols = c * HEAD_DIM + np.arange(HEAD_DIM)
        wq_c = np.ascontiguousarray(Wq[:, qcols]).astype(NP_BF16)
        wkv_c = np.ascontiguousarray(
            np.concatenate([Wk[:, kcols], Wv[:, vcols]], axis=1)).astype(NP_BF16)
        wo_c = np.ascontiguousarray(
            Wo[c * N_REP * HEAD_DIM:(c + 1) * N_REP * HEAD_DIM, :]).astype(NP_BF16)
        in_maps.append({
            "xT": xT, "wq": wq_c, "wkv": wkv_c, "wo": wo_c,
            "cos2": cos2, "sin2n": sin2n, "dmask": dmask,
            "ident": ident, "ones_c": ones_c, "ones_r": ones_r,
        })
    return in_maps


def get_nc():
    if "nc" not in _CACHED:
        _CACHED["nc"] = _build_nc()
    return _CACHED["nc"]


def kernel(x, start_pos, freqs_cos, freqs_sin, mask, cache_k, cache_v,
           Wq, Wk, Wv, Wo, _trace=False, _tmpdir=None):
    assert int(start_pos) == 0, "kernel hardcodes start_pos == 0"
    nc = get_nc()
    in_maps = _prep_inputs(x, freqs_cos, freqs_sin, Wq, Wk, Wv, Wo)
    kwargs = {}
    if _trace:
        kwargs = {"trace": True, "tmpdir": _tmpdir}
    res = run_bass_kernel_spmd(nc, in_maps, core_ids=list(range(N_CORES)), **kwargs)
    out = res.results[0]["y"].astype(np.float64)
    for c in range(1, N_CORES):
        out += res.results[c]["y"]
    out = out.astype(np.float32)
    if _trace:
        return out, res
    return out
